# revision 3
# baseline (speedup 1.0000x reference)
"""Trainium2 Bass kernel for nn_AsyncConvBis (geodesic patch conv / GNN message passing).

Reference computation, per batch b and vertex v:
    patches[r, jj, c] = y[b, vert[b, v, r, jj], c]            (gather 3x16 neighbors)
    z[d, f] = sum_{r, jj, c} patches[r, jj, c] * kernel[r, (jj - d) % 16, c, f]
    z += y[b, v] @ center_kernel + bias
    out[b, v, f] = max_d relu(z[d, f])

Key restructuring (same math as the ap_gather variant):
  - relu and max_d commute and center/bias are d-independent, so everything folds
    into accumulated matmuls per 128-vertex subtile against the block-circulant
        Wconv[(j, c), (d, f)] = kernel[j//16, (j%16 - d) % 16, c, f]
    plus a small center/bias chunk [y[v], 1] @ [center_kernel; bias].

  - Gather via SDMA-engine dma_gather (SWDGE, 4 queues) from an HBM table of
    256B rows (y[u] padded 32->128 bf16). Non-transpose mode places index i at
    SBUF [partition i%128, row i//128]. Per 2-subtile batch: 12 calls of 1024
    indices (one per 4-slot quad), ordered so G rows come out as
    (quad k, subtile t2, slot-member g):
        G[p, k*8 + t2*4 + g, 0:32] = y[vert[v(t2,p), slot 4k+g], :]
    A DVE copy drops the 96-column pad (Gt [128, rows, 32]) and one xbar
    dma_start_transpose per batch re-blocks it into
        M[p, k*2 + t2, v] = Gt[v, 4*(k*2+t2) + p//32, p%32]
    i.e. M[:, 2k+t2, :] is exactly the K=128 lhsT (4 slots x 32 channels on
    partitions, 128 vertices on columns) for quad k of subtile t2.

  - Per subtile: 12 quad chunks + 1 center chunk -> 26 matmuls (N=512,
    Z[128v, 1024df] in PSUM) -> DVE max-reduce over d -> relu -> store.

This trades the Q7 ap_gather (33 cyc/index, ~1.04 ms/core) for the SDMA
engines (~2.8 ns/index at 4 queues with deep buffering), keeping GpSimd free
of everything except descriptor generation.

Sharding: batch-major over flattened (b, v): cores 0-3 handle batch 0, cores 4-7
batch 1, each owning 6250 consecutive vertices (padded to 6272 = 49 subtiles).
No collectives needed.

Self-contained: hardcodes all shapes; host-side work is limited to sharding,
layout/dtype transforms of inputs, and building W from kernel/center_kernel/bias.
"""

import numpy as np
import ml_dtypes

import concourse.bass as bass
import concourse.bacc as bacc
import concourse.tile as tile
import concourse.mybir as mybir
from concourse.bass_utils import run_bass_kernel_spmd

# Problem shapes
B, NV, C = 2, 25000, 32
NR, ND, F = 3, 16, 64
NCORES = 8
VPC = (B * NV) // NCORES          # 6250 vertices per core
SUB = 128                         # vertices per subtile
NSUB = (VPC + SUB - 1) // SUB     # 49
NPAD = NSUB * SUB                 # 6272
NSLOT = NR * ND                   # 48 conv slots
NQ = NSLOT // 4                   # 12 quads of 4 slots
NDF = ND * F                      # 1024
ES = 128                          # gather row: 32 channels + 96 pad (256B)
NQUEUES = 4
BATCHES = [2] * 24 + [1]          # subtiles per gather batch (sum = NSUB)

_DT = mybir.dt
BF16 = ml_dtypes.bfloat16


def build_graph():
    """Build the per-core SPMD Bass graph (identical on all 8 cores)."""
    nc = bacc.Bacc("TRN2", target_bir_lowering=False, num_swdge_queues=NQUEUES)

    ytab = nc.dram_tensor("ytab", [NV, ES], _DT.bfloat16, kind="ExternalInput")
    ycen = nc.dram_tensor("ycen", [33, NPAD], _DT.bfloat16, kind="ExternalInput")
    idx_cols = sum(NQ * bs * 4 * SUB // 16 for bs in BATCHES)   # 64 cols/call (bs=2)
    idx = nc.dram_tensor("idx", [128, idx_cols], _DT.int16, kind="ExternalInput")
    w = nc.dram_tensor("w", [128, NQ * NDF], _DT.bfloat16, kind="ExternalInput")
    wcb = nc.dram_tensor("wcb", [33, NDF], _DT.bfloat16, kind="ExternalInput")
    out = nc.dram_tensor("out", [NPAD, F], _DT.float32, kind="ExternalOutput")

    with tile.TileContext(nc) as tc:
        with (
            tc.tile_pool(name="const", bufs=1) as const_pool,
            tc.tile_pool(name="gp", bufs=3) as gpool,
            tc.tile_pool(name="mp", bufs=3) as mpool,
            tc.tile_pool(name="ip", bufs=3) as ipool,
            tc.tile_pool(name="res", bufs=4) as rpool,
            tc.tile_pool(name="psum", bufs=4, space="PSUM") as pspool,
        ):
            wsb = const_pool.tile([128, NQ, NDF], _DT.bfloat16)
            nc.sync.dma_start(wsb[:].rearrange("p a b -> p (a b)"), w[:])
            wcbsb = const_pool.tile([33, NDF], _DT.bfloat16)
            nc.sync.dma_start(wcbsb[:], wcb[:])
            ycsb = const_pool.tile([33, NPAD], _DT.bfloat16)
            nc.sync.dma_start(ycsb[:], ycen[:])

            ioff = 0   # running column offset into idx
            t0 = 0     # running subtile index
            qn = 0     # round-robin SWDGE queue counter
            for bs in BATCHES:
                nrows = NQ * bs * 4           # G rows this batch (96 / 48)
                ni = bs * 4 * SUB             # indices per call (1024 / 512)
                iw = ni // 16                 # idx cols per call
                isb = ipool.tile([128, NQ, iw], _DT.int16, tag="isb")
                nc.sync.dma_start(
                    isb[:].rearrange("p a b -> p (a b)"),
                    idx[:, ioff : ioff + NQ * iw],
                )
                g = gpool.tile([128, nrows, ES], _DT.bfloat16, tag="g")
                for k in range(NQ):
                    nc.gpsimd.dma_gather(
                        g[:, k * bs * 4 : (k + 1) * bs * 4, :],
                        ytab[:, :],
                        isb[:, k, :],
                        ni, ni, ES,
                        transpose=False,
                        queue_num=qn % NQUEUES,
                    )
                    qn += 1
                ioff += NQ * iw

                gt = gpool.tile([128, nrows, C], _DT.bfloat16, tag="gt")
                nc.vector.tensor_copy(gt[:], g[:, :, 0:C])
                m = mpool.tile([128, nrows // 4, SUB], _DT.bfloat16, tag="m")
                nc.sync.dma_start_transpose(m[:], gt[:].rearrange("p a b -> p (a b)"))

                for t2 in range(bs):
                    t = t0 + t2
                    ps = pspool.tile([128, NDF], _DT.float32)
                    for k in range(NQ):
                        lhsT = m[:, k * bs + t2, :]
                        nc.tensor.matmul(
                            ps[:, 0:512], lhsT=lhsT, rhs=wsb[:, k, 0:512],
                            start=(k == 0), stop=False,
                        )
                        nc.tensor.matmul(
                            ps[:, 512:1024], lhsT=lhsT, rhs=wsb[:, k, 512:1024],
                            start=(k == 0), stop=False,
                        )
                    lhsT = ycsb[:, t * SUB : (t + 1) * SUB]
                    nc.tensor.matmul(
                        ps[:, 0:512], lhsT=lhsT, rhs=wcbsb[:, 0:512],
                        start=False, stop=True,
                    )
                    nc.tensor.matmul(
                        ps[:, 512:1024], lhsT=lhsT, rhs=wcbsb[:, 512:1024],
                        start=False, stop=True,
                    )

                    r = rpool.tile([128, F], _DT.float32, tag="r")
                    nc.vector.tensor_reduce(
                        out=r[:],
                        in_=ps[:].rearrange("p (d f) -> p f d", d=ND),
                        axis=mybir.AxisListType.X,
                        op=mybir.AluOpType.max,
                    )
                    rr = rpool.tile([128, F], _DT.float32, tag="rr")
                    nc.vector.tensor_scalar_max(rr[:], r[:], 0.0)
                    nc.sync.dma_start(out[t * SUB : (t + 1) * SUB, :], rr[:])
                t0 += bs

    nc.compile()
    return nc


def _build_w(kernel):
    """Quad-ordered conv weights [128, NQ*NDF].

    wsb[p, k, n] = Wconv[slot 4k + p//32, channel p%32, n]
    with Wconv[j, c, (d,f)] = kernel[j//16, (j%16 - d) % 16, c, f].
    """
    kernel = np.asarray(kernel, dtype=np.float32)
    jj = np.arange(ND)
    d = np.arange(ND)
    dd = (jj[:, None] - d[None, :]) % ND         # [jj, d]
    wconv = kernel[:, dd, :, :]                  # [NR, jj, d, C, F]
    wconv = wconv.transpose(0, 1, 3, 2, 4).reshape(NSLOT, C, NDF)  # [j, c, n]
    p = np.arange(128)
    wp = np.empty((128, NQ, NDF), dtype=np.float32)
    for k in range(NQ):
        wp[:, k, :] = wconv[4 * k + p // 32, p % 32, :]
    return np.ascontiguousarray(wp.reshape(128, NQ * NDF).astype(BF16))


def _build_wcb(center_kernel, bias):
    wcb = np.empty((33, NDF), dtype=np.float32)
    wcb[:32] = np.broadcast_to(
        np.asarray(center_kernel, np.float32)[:, None, :], (C, ND, F)
    ).reshape(C, NDF)
    wcb[32] = np.broadcast_to(np.asarray(bias, np.float32)[None, :], (ND, F)).reshape(NDF)
    return np.ascontiguousarray(wcb.astype(BF16))


def _build_ytab(yb):
    """HBM gather table [NV, 128] bf16: row u = [y[u] (32ch) | zeros(96)]."""
    t = np.zeros((NV, ES), dtype=BF16)
    t[:, :C] = yb.astype(BF16)
    return np.ascontiguousarray(t)


def _build_ycen(yb, v0):
    """Center/bias operand [33, NPAD]: rows 0-31 own-slab channels, row 32 ones."""
    yc = np.zeros((33, NPAD), dtype=np.float32)
    yc[:32, :VPC] = yb[v0 : v0 + VPC].T
    yc[32, :] = 1.0
    return np.ascontiguousarray(yc.astype(BF16))


def _build_idx(vert_b, v0):
    """Wrapped int16 gather indices [128, idx_cols] for one core's slab.

    Per batch (bs subtiles), per quad call k: flat gather position
    i = (t2*4 + g)*128 + p  ->  index vert[v0 + (t0+t2)*128 + p, slot 4k+g],
    wrapped as idx[i%16, ioff + i//16] and replicated across the 8 Q7 groups
    (dma_gather reads partitions 0-15).
    """
    slots = np.zeros((NPAD, NSLOT), dtype=np.int64)
    slots[:VPC] = vert_b[v0 : v0 + VPC].reshape(VPC, NSLOT)
    cols = []
    t0 = 0
    for bs in BATCHES:
        ni = bs * 4 * SUB
        # flat[i] for call k: i = (t2*4+g)*128 + p
        S = slots[t0 * SUB : (t0 + bs) * SUB].reshape(bs, SUB, NSLOT)
        blk = np.empty((NQ, 128, ni // 16), dtype=np.int16)
        for k in range(NQ):
            flat = np.empty(ni, dtype=np.int16)
            for t2 in range(bs):
                for gmem in range(4):
                    r = t2 * 4 + gmem
                    flat[r * SUB : (r + 1) * SUB] = S[t2, :, 4 * k + gmem]
            wrapped = flat.reshape(ni // 16, 16).T          # [16, ni/16]
            blk[k] = np.tile(wrapped, (8, 1))
        cols.append(blk.transpose(1, 0, 2).reshape(128, NQ * (ni // 16)))
        t0 += bs
    return np.ascontiguousarray(np.concatenate(cols, axis=1))


_NC_CACHE = None
_LAST_IN_MAPS = None


def _host_fallback(y, exp_map, kernel, center_kernel, bias):
    """Numpy reference path; only used if exp_map's batch column is nonstandard."""
    patches = y[exp_map[..., 0], exp_map[..., 1]]        # [B, NV, NR, ND, C]
    jj = np.arange(ND)
    d = np.arange(ND)
    wk = kernel[:, (jj[:, None] - d[None, :]) % ND]      # [NR, jj, d, C, F]
    z = np.einsum("bvrjc,rjdcf->bvdf", patches, wk, optimize=True)
    z = z + (y @ center_kernel)[:, :, None, :] + bias
    return np.max(np.maximum(z, 0.0), axis=2).astype(np.float32)


def kernel(y, exp_map, kernel, center_kernel, bias):
    global _NC_CACHE, _LAST_IN_MAPS
    y = np.asarray(y, dtype=np.float32)
    exp_map = np.asarray(exp_map)
    bcast = np.arange(B, dtype=exp_map.dtype)[:, None, None, None]
    if not np.array_equal(exp_map[..., 0], np.broadcast_to(bcast, exp_map.shape[:-1])):
        return _host_fallback(y, exp_map, np.asarray(kernel, np.float32),
                              np.asarray(center_kernel, np.float32),
                              np.asarray(bias, np.float32))
    vert = np.ascontiguousarray(exp_map[..., 1]).astype(np.int64)  # [B, NV, NR, ND]

    wp = _build_w(kernel)
    wcb = _build_wcb(center_kernel, bias)
    tabs = [_build_ytab(y[b]) for b in range(B)]

    in_maps = []
    for core in range(NCORES):
        b = core // (NCORES // B)
        v0 = (core % (NCORES // B)) * VPC
        in_maps.append(
            {
                "ytab": tabs[b],
                "ycen": _build_ycen(y[b], v0),
                "idx": _build_idx(vert[b], v0),
                "w": wp,
                "wcb": wcb,
            }
        )

    if _NC_CACHE is None:
        _NC_CACHE = build_graph()
    nc = _NC_CACHE
    _LAST_IN_MAPS = in_maps

    res = run_bass_kernel_spmd(nc, in_maps, core_ids=list(range(NCORES)))
    outs = [res.results[i]["out"][:VPC] for i in range(NCORES)]
    full = np.concatenate(outs, axis=0).reshape(B, NV, F).astype(np.float32)
    return full


if __name__ == "__main__":
    rng = np.random.default_rng(0)
    y = rng.standard_normal((B, NV, C), dtype=np.float32)
    vert = rng.integers(0, NV, size=(B, NV, NR, ND), dtype=np.int32)
    bidx = np.broadcast_to(np.arange(B, dtype=np.int32)[:, None, None, None], vert.shape)
    exp_map = np.stack([bidx, vert], axis=-1)
    kern = rng.standard_normal((NR, ND, C, F), dtype=np.float32) * 0.05
    ck = rng.standard_normal((C, F), dtype=np.float32) * 0.05
    bs = np.zeros((F,), dtype=np.float32)
    out = kernel(y=y, exp_map=exp_map, kernel=kern, center_kernel=ck, bias=bs)
    print("out", out.shape, out.dtype, float(out.mean()))


# revision 4
# speedup vs baseline: 1.1606x; 1.1606x over previous
"""Trainium2 Bass kernel for nn_AsyncConvBis (geodesic patch conv / GNN message passing).

Reference computation, per batch b and vertex v:
    patches[r, jj, c] = y[b, vert[b, v, r, jj], c]            (gather 3x16 neighbors)
    z[d, f] = sum_{r, jj, c} patches[r, jj, c] * kernel[r, (jj - d) % 16, c, f]
    z += y[b, v] @ center_kernel + bias
    out[b, v, f] = max_d relu(z[d, f])

Key restructuring (same math as the ap_gather variant):
  - relu and max_d commute and center/bias are d-independent, so everything folds
    into accumulated matmuls per 128-vertex subtile against the block-circulant
        Wconv[(j, c), (d, f)] = kernel[j//16, (j%16 - d) % 16, c, f]
    plus a small center/bias chunk [y[v], 1] @ [center_kernel; bias].

  - Gather via SDMA-engine dma_gather (SWDGE, 4 queues) from an HBM table of
    256B rows (y[u] padded 32->128 bf16). Non-transpose mode places index i at
    SBUF [partition i%128, row i//128]. Per 2-subtile batch: 12 calls of 1024
    indices (one per 4-slot quad), ordered so G rows come out as
    (quad k, subtile t2, slot-member g):
        G[p, k*8 + t2*4 + g, 0:32] = y[vert[v(t2,p), slot 4k+g], :]
    A DVE copy drops the 96-column pad (Gt [128, rows, 32]) and one xbar
    dma_start_transpose per batch re-blocks it into
        M[p, k*2 + t2, v] = Gt[v, 4*(k*2+t2) + p//32, p%32]
    i.e. M[:, 2k+t2, :] is exactly the K=128 lhsT (4 slots x 32 channels on
    partitions, 128 vertices on columns) for quad k of subtile t2.

  - Per subtile: 12 quad chunks + 1 center chunk -> 26 matmuls (N=512,
    Z[128v, 1024df] in PSUM) -> DVE max-reduce over d -> relu -> store.

This trades the Q7 ap_gather (33 cyc/index, ~1.04 ms/core) for the SDMA
engines (~2.8 ns/index at 4 queues with deep buffering), keeping GpSimd free
of everything except descriptor generation.

Sharding: batch-major over flattened (b, v): cores 0-3 handle batch 0, cores 4-7
batch 1, each owning 6250 consecutive vertices (padded to 6272 = 49 subtiles).
No collectives needed.

Self-contained: hardcodes all shapes; host-side work is limited to sharding,
layout/dtype transforms of inputs, and building W from kernel/center_kernel/bias.
"""

import numpy as np
import ml_dtypes

import concourse.bass as bass
import concourse.bacc as bacc
import concourse.tile as tile
import concourse.mybir as mybir
from concourse.bass_utils import run_bass_kernel_spmd

# Problem shapes
B, NV, C = 2, 25000, 32
NR, ND, F = 3, 16, 64
NCORES = 8
VPC = (B * NV) // NCORES          # 6250 vertices per core
SUB = 128                         # vertices per subtile
NSUB = (VPC + SUB - 1) // SUB     # 49
NPAD = NSUB * SUB                 # 6272
NSLOT = NR * ND                   # 48 conv slots
NQ = NSLOT // 4                   # 12 quads of 4 slots
NDF = ND * F                      # 1024
ES = 128                          # gather row: 32 channels + 96 pad (256B)
NQUEUES = 4
BATCHES = [2] * 24 + [1]          # subtiles per gather batch (sum = NSUB)

_DT = mybir.dt
BF16 = ml_dtypes.bfloat16


def build_graph():
    """Build the per-core SPMD Bass graph (identical on all 8 cores)."""
    nc = bacc.Bacc("TRN2", target_bir_lowering=False, num_swdge_queues=NQUEUES)

    ytab = nc.dram_tensor("ytab", [NV, ES], _DT.bfloat16, kind="ExternalInput")
    ycen = nc.dram_tensor("ycen", [33, NPAD], _DT.bfloat16, kind="ExternalInput")
    idx_cols = sum(NQ * bs * 4 * SUB // 16 for bs in BATCHES)   # 64 cols/call (bs=2)
    idx = nc.dram_tensor("idx", [128, idx_cols], _DT.int16, kind="ExternalInput")
    w = nc.dram_tensor("w", [128, NQ * NDF], _DT.bfloat16, kind="ExternalInput")
    wcb = nc.dram_tensor("wcb", [33, NDF], _DT.bfloat16, kind="ExternalInput")
    out = nc.dram_tensor("out", [NPAD, F], _DT.float32, kind="ExternalOutput")

    with tile.TileContext(nc) as tc:
        with (
            tc.tile_pool(name="const", bufs=1) as const_pool,
            tc.tile_pool(name="gp", bufs=4) as gpool,
            tc.tile_pool(name="mp", bufs=4) as mpool,
            tc.tile_pool(name="ip", bufs=4) as ipool,
            tc.tile_pool(name="res", bufs=4) as rpool,
            tc.tile_pool(name="psum", bufs=4, space="PSUM") as pspool,
        ):
            wsb = const_pool.tile([128, NQ, NDF], _DT.bfloat16)
            nc.sync.dma_start(wsb[:].rearrange("p a b -> p (a b)"), w[:])
            wcbsb = const_pool.tile([33, NDF], _DT.bfloat16)
            nc.sync.dma_start(wcbsb[:], wcb[:])
            ycsb = const_pool.tile([33, NPAD], _DT.bfloat16)
            nc.sync.dma_start(ycsb[:], ycen[:])

            ioff = 0   # running column offset into idx
            t0 = 0     # running subtile index
            qn = 0     # round-robin SWDGE queue counter
            for bs in BATCHES:
                nrows = NQ * bs * 4           # G rows this batch (96 / 48)
                ni = bs * 4 * SUB             # indices per call (1024 / 512)
                iw = ni // 16                 # idx cols per call
                isb = ipool.tile([128, NQ, iw], _DT.int16, tag="isb")
                nc.sync.dma_start(
                    isb[:].rearrange("p a b -> p (a b)"),
                    idx[:, ioff : ioff + NQ * iw],
                )
                g = gpool.tile([128, nrows, ES], _DT.bfloat16, tag="g")
                for k in range(NQ):
                    nc.gpsimd.dma_gather(
                        g[:, k * bs * 4 : (k + 1) * bs * 4, :],
                        ytab[:, :],
                        isb[:, k, :],
                        ni, ni, ES,
                        transpose=False,
                        queue_num=qn % NQUEUES,
                    )
                    qn += 1
                ioff += NQ * iw

                gt = gpool.tile([128, nrows, C], _DT.bfloat16, tag="gt")
                nc.vector.tensor_copy(gt[:], g[:, :, 0:C])
                m = mpool.tile([128, nrows // 4, SUB], _DT.bfloat16, tag="m")
                nc.sync.dma_start_transpose(m[:], gt[:].rearrange("p a b -> p (a b)"))

                for t2 in range(bs):
                    t = t0 + t2
                    ps = pspool.tile([128, NDF], _DT.float32)
                    for k in range(NQ):
                        lhsT = m[:, k * bs + t2, :]
                        nc.tensor.matmul(
                            ps[:, 0:512], lhsT=lhsT, rhs=wsb[:, k, 0:512],
                            start=(k == 0), stop=False,
                        )
                        nc.tensor.matmul(
                            ps[:, 512:1024], lhsT=lhsT, rhs=wsb[:, k, 512:1024],
                            start=(k == 0), stop=False,
                        )
                    lhsT = ycsb[:, t * SUB : (t + 1) * SUB]
                    nc.tensor.matmul(
                        ps[:, 0:512], lhsT=lhsT, rhs=wcbsb[:, 0:512],
                        start=False, stop=True,
                    )
                    nc.tensor.matmul(
                        ps[:, 512:1024], lhsT=lhsT, rhs=wcbsb[:, 512:1024],
                        start=False, stop=True,
                    )

                    r = rpool.tile([128, F], _DT.float32, tag="r")
                    nc.vector.tensor_reduce(
                        out=r[:],
                        in_=ps[:].rearrange("p (d f) -> p f d", d=ND),
                        axis=mybir.AxisListType.X,
                        op=mybir.AluOpType.max,
                    )
                    rr = rpool.tile([128, F], _DT.float32, tag="rr")
                    nc.vector.tensor_scalar_max(rr[:], r[:], 0.0)
                    nc.sync.dma_start(out[t * SUB : (t + 1) * SUB, :], rr[:])
                t0 += bs

    nc.compile()
    return nc


def _build_w(kernel):
    """Quad-ordered conv weights [128, NQ*NDF].

    wsb[p, k, n] = Wconv[slot 4k + p//32, channel p%32, n]
    with Wconv[j, c, (d,f)] = kernel[j//16, (j%16 - d) % 16, c, f].
    """
    kernel = np.asarray(kernel, dtype=np.float32)
    jj = np.arange(ND)
    d = np.arange(ND)
    dd = (jj[:, None] - d[None, :]) % ND         # [jj, d]
    wconv = kernel[:, dd, :, :]                  # [NR, jj, d, C, F]
    wconv = wconv.transpose(0, 1, 3, 2, 4).reshape(NSLOT, C, NDF)  # [j, c, n]
    p = np.arange(128)
    wp = np.empty((128, NQ, NDF), dtype=np.float32)
    for k in range(NQ):
        wp[:, k, :] = wconv[4 * k + p // 32, p % 32, :]
    return np.ascontiguousarray(wp.reshape(128, NQ * NDF).astype(BF16))


def _build_wcb(center_kernel, bias):
    wcb = np.empty((33, NDF), dtype=np.float32)
    wcb[:32] = np.broadcast_to(
        np.asarray(center_kernel, np.float32)[:, None, :], (C, ND, F)
    ).reshape(C, NDF)
    wcb[32] = np.broadcast_to(np.asarray(bias, np.float32)[None, :], (ND, F)).reshape(NDF)
    return np.ascontiguousarray(wcb.astype(BF16))


def _build_ytab(yb):
    """HBM gather table [NV, 128] bf16: row u = [y[u] (32ch) | zeros(96)]."""
    t = np.zeros((NV, ES), dtype=BF16)
    t[:, :C] = yb.astype(BF16)
    return np.ascontiguousarray(t)


def _build_ycen(yb, v0):
    """Center/bias operand [33, NPAD]: rows 0-31 own-slab channels, row 32 ones."""
    yc = np.zeros((33, NPAD), dtype=np.float32)
    yc[:32, :VPC] = yb[v0 : v0 + VPC].T
    yc[32, :] = 1.0
    return np.ascontiguousarray(yc.astype(BF16))


def _build_idx(vert_b, v0):
    """Wrapped int16 gather indices [128, idx_cols] for one core's slab.

    Per batch (bs subtiles), per quad call k: flat gather position
    i = (t2*4 + g)*128 + p  ->  index vert[v0 + (t0+t2)*128 + p, slot 4k+g],
    wrapped as idx[i%16, ioff + i//16] and replicated across the 8 Q7 groups
    (dma_gather reads partitions 0-15).
    """
    slots = np.zeros((NPAD, NSLOT), dtype=np.int64)
    slots[:VPC] = vert_b[v0 : v0 + VPC].reshape(VPC, NSLOT)
    cols = []
    t0 = 0
    for bs in BATCHES:
        ni = bs * 4 * SUB
        # flat[i] for call k: i = (t2*4+g)*128 + p
        S = slots[t0 * SUB : (t0 + bs) * SUB].reshape(bs, SUB, NSLOT)
        blk = np.empty((NQ, 128, ni // 16), dtype=np.int16)
        for k in range(NQ):
            flat = np.empty(ni, dtype=np.int16)
            for t2 in range(bs):
                for gmem in range(4):
                    r = t2 * 4 + gmem
                    flat[r * SUB : (r + 1) * SUB] = S[t2, :, 4 * k + gmem]
            wrapped = flat.reshape(ni // 16, 16).T          # [16, ni/16]
            blk[k] = np.tile(wrapped, (8, 1))
        cols.append(blk.transpose(1, 0, 2).reshape(128, NQ * (ni // 16)))
        t0 += bs
    return np.ascontiguousarray(np.concatenate(cols, axis=1))


_NC_CACHE = None
_LAST_IN_MAPS = None


def _host_fallback(y, exp_map, kernel, center_kernel, bias):
    """Numpy reference path; only used if exp_map's batch column is nonstandard."""
    patches = y[exp_map[..., 0], exp_map[..., 1]]        # [B, NV, NR, ND, C]
    jj = np.arange(ND)
    d = np.arange(ND)
    wk = kernel[:, (jj[:, None] - d[None, :]) % ND]      # [NR, jj, d, C, F]
    z = np.einsum("bvrjc,rjdcf->bvdf", patches, wk, optimize=True)
    z = z + (y @ center_kernel)[:, :, None, :] + bias
    return np.max(np.maximum(z, 0.0), axis=2).astype(np.float32)


def kernel(y, exp_map, kernel, center_kernel, bias):
    global _NC_CACHE, _LAST_IN_MAPS
    y = np.asarray(y, dtype=np.float32)
    exp_map = np.asarray(exp_map)
    bcast = np.arange(B, dtype=exp_map.dtype)[:, None, None, None]
    if not np.array_equal(exp_map[..., 0], np.broadcast_to(bcast, exp_map.shape[:-1])):
        return _host_fallback(y, exp_map, np.asarray(kernel, np.float32),
                              np.asarray(center_kernel, np.float32),
                              np.asarray(bias, np.float32))
    vert = np.ascontiguousarray(exp_map[..., 1]).astype(np.int64)  # [B, NV, NR, ND]

    wp = _build_w(kernel)
    wcb = _build_wcb(center_kernel, bias)
    tabs = [_build_ytab(y[b]) for b in range(B)]

    in_maps = []
    for core in range(NCORES):
        b = core // (NCORES // B)
        v0 = (core % (NCORES // B)) * VPC
        in_maps.append(
            {
                "ytab": tabs[b],
                "ycen": _build_ycen(y[b], v0),
                "idx": _build_idx(vert[b], v0),
                "w": wp,
                "wcb": wcb,
            }
        )

    if _NC_CACHE is None:
        _NC_CACHE = build_graph()
    nc = _NC_CACHE
    _LAST_IN_MAPS = in_maps

    res = run_bass_kernel_spmd(nc, in_maps, core_ids=list(range(NCORES)))
    outs = [res.results[i]["out"][:VPC] for i in range(NCORES)]
    full = np.concatenate(outs, axis=0).reshape(B, NV, F).astype(np.float32)
    return full


if __name__ == "__main__":
    rng = np.random.default_rng(0)
    y = rng.standard_normal((B, NV, C), dtype=np.float32)
    vert = rng.integers(0, NV, size=(B, NV, NR, ND), dtype=np.int32)
    bidx = np.broadcast_to(np.arange(B, dtype=np.int32)[:, None, None, None], vert.shape)
    exp_map = np.stack([bidx, vert], axis=-1)
    kern = rng.standard_normal((NR, ND, C, F), dtype=np.float32) * 0.05
    ck = rng.standard_normal((C, F), dtype=np.float32) * 0.05
    bs = np.zeros((F,), dtype=np.float32)
    out = kernel(y=y, exp_map=exp_map, kernel=kern, center_kernel=ck, bias=bs)
    print("out", out.shape, out.dtype, float(out.mean()))


# revision 7
# speedup vs baseline: 1.1613x; 1.0006x over previous
"""Trainium2 Bass kernel for nn_AsyncConvBis (geodesic patch conv / GNN message passing).

Reference computation, per batch b and vertex v:
    patches[r, jj, c] = y[b, vert[b, v, r, jj], c]            (gather 3x16 neighbors)
    z[d, f] = sum_{r, jj, c} patches[r, jj, c] * kernel[r, (jj - d) % 16, c, f]
    z += y[b, v] @ center_kernel + bias
    out[b, v, f] = max_d relu(z[d, f])

Key restructuring (same math as the ap_gather variant):
  - relu and max_d commute and center/bias are d-independent, so everything folds
    into accumulated matmuls per 128-vertex subtile against the block-circulant
        Wconv[(j, c), (d, f)] = kernel[j//16, (j%16 - d) % 16, c, f]
    plus a small center/bias chunk [y[v], 1] @ [center_kernel; bias].

  - Gather via SDMA-engine dma_gather (SWDGE, 4 queues) from an HBM table of
    256B rows (y[u] padded 32->128 bf16). Non-transpose mode places index i at
    SBUF [partition i%128, row i//128]. Per 2-subtile batch: 12 calls of 1024
    indices (one per 4-slot quad), ordered so G rows come out as
    (quad k, subtile t2, slot-member g):
        G[p, k*8 + t2*4 + g, 0:32] = y[vert[v(t2,p), slot 4k+g], :]
    A DVE copy drops the 96-column pad (Gt [128, rows, 32]) and one xbar
    dma_start_transpose per batch re-blocks it into
        M[p, k*2 + t2, v] = Gt[v, 4*(k*2+t2) + p//32, p%32]
    i.e. M[:, 2k+t2, :] is exactly the K=128 lhsT (4 slots x 32 channels on
    partitions, 128 vertices on columns) for quad k of subtile t2.

  - Per subtile: 12 quad chunks + 1 center chunk -> 26 matmuls (N=512,
    Z[128v, 1024df] in PSUM) -> DVE max-reduce over d -> relu -> store.

This trades the Q7 ap_gather (33 cyc/index, ~1.04 ms/core) for the SDMA
engines (~2.8 ns/index at 4 queues with deep buffering), keeping GpSimd free
of everything except descriptor generation.

Sharding: batch-major over flattened (b, v): cores 0-3 handle batch 0, cores 4-7
batch 1, each owning 6250 consecutive vertices (padded to 6272 = 49 subtiles).
No collectives needed.

Self-contained: hardcodes all shapes; host-side work is limited to sharding,
layout/dtype transforms of inputs, and building W from kernel/center_kernel/bias.
"""

import numpy as np
import ml_dtypes

import concourse.bass as bass
import concourse.bacc as bacc
import concourse.tile as tile
import concourse.mybir as mybir
from concourse.bass_utils import run_bass_kernel_spmd

# Problem shapes
B, NV, C = 2, 25000, 32
NR, ND, F = 3, 16, 64
NCORES = 8
VPC = (B * NV) // NCORES          # 6250 vertices per core
SUB = 128                         # vertices per subtile
NSUB = (VPC + SUB - 1) // SUB     # 49
NPAD = NSUB * SUB                 # 6272
NSLOT = NR * ND                   # 48 conv slots
NQ = NSLOT // 4                   # 12 quads of 4 slots
NDF = ND * F                      # 1024
ES = 128                          # gather row: 32 channels + 96 pad (256B)
NQUEUES = 4
BATCHES = [2] * 24 + [1]          # subtiles per gather batch (sum = NSUB)

_DT = mybir.dt
BF16 = ml_dtypes.bfloat16


def build_graph():
    """Build the per-core SPMD Bass graph (identical on all 8 cores)."""
    nc = bacc.Bacc("TRN2", target_bir_lowering=False, num_swdge_queues=NQUEUES)

    ytab = nc.dram_tensor("ytab", [NV, ES], _DT.bfloat16, kind="ExternalInput")
    ycen = nc.dram_tensor("ycen", [33, NPAD], _DT.bfloat16, kind="ExternalInput")
    idx_cols = sum(NQ * bs * 4 * SUB // 16 for bs in BATCHES)   # 64 cols/call (bs=2)
    idx = nc.dram_tensor("idx", [128, idx_cols], _DT.int16, kind="ExternalInput")
    w = nc.dram_tensor("w", [128, NQ * NDF], _DT.bfloat16, kind="ExternalInput")
    wcb = nc.dram_tensor("wcb", [33, NDF], _DT.bfloat16, kind="ExternalInput")
    out = nc.dram_tensor("out", [NPAD, F], _DT.float32, kind="ExternalOutput")

    with tile.TileContext(nc) as tc:
        with (
            tc.tile_pool(name="const", bufs=1) as const_pool,
            tc.tile_pool(name="gp", bufs=4) as gpool,
            tc.tile_pool(name="mp", bufs=5) as mpool,
            tc.tile_pool(name="ip", bufs=4) as ipool,
            tc.tile_pool(name="res", bufs=4) as rpool,
            tc.tile_pool(name="psum", bufs=4, space="PSUM") as pspool,
        ):
            wsb = const_pool.tile([128, NQ, NDF], _DT.bfloat16)
            nc.sync.dma_start(wsb[:].rearrange("p a b -> p (a b)"), w[:])
            wcbsb = const_pool.tile([33, NDF], _DT.bfloat16)
            nc.sync.dma_start(wcbsb[:], wcb[:])
            ycsb = const_pool.tile([33, NPAD], _DT.bfloat16)
            nc.sync.dma_start(ycsb[:], ycen[:])

            ioff = 0   # running column offset into idx
            t0 = 0     # running subtile index
            qn = 0     # round-robin SWDGE queue counter
            for bs in BATCHES:
                nrows = NQ * bs * 4           # G rows this batch (96 / 48)
                ni = bs * 4 * SUB             # indices per call (1024 / 512)
                iw = ni // 16                 # idx cols per call
                isb = ipool.tile([128, NQ, iw], _DT.int16, tag="isb")
                nc.sync.dma_start(
                    isb[:].rearrange("p a b -> p (a b)"),
                    idx[:, ioff : ioff + NQ * iw],
                )
                g = gpool.tile([128, nrows, ES], _DT.bfloat16, tag="g")
                for k in range(NQ):
                    nc.gpsimd.dma_gather(
                        g[:, k * bs * 4 : (k + 1) * bs * 4, :],
                        ytab[:, :],
                        isb[:, k, :],
                        ni, ni, ES,
                        transpose=False,
                        queue_num=qn % NQUEUES,
                    )
                    qn += 1
                ioff += NQ * iw

                gt = gpool.tile([128, nrows, C], _DT.bfloat16, tag="gt")
                nc.vector.tensor_copy(gt[:], g[:, :, 0:C])
                m = mpool.tile([128, nrows // 4, SUB], _DT.bfloat16, tag="m")
                nc.sync.dma_start_transpose(m[:], gt[:].rearrange("p a b -> p (a b)"))

                for t2 in range(bs):
                    t = t0 + t2
                    ps = pspool.tile([128, NDF], _DT.float32)
                    for k in range(NQ):
                        lhsT = m[:, k * bs + t2, :]
                        nc.tensor.matmul(
                            ps[:, 0:512], lhsT=lhsT, rhs=wsb[:, k, 0:512],
                            start=(k == 0), stop=False,
                        )
                        nc.tensor.matmul(
                            ps[:, 512:1024], lhsT=lhsT, rhs=wsb[:, k, 512:1024],
                            start=(k == 0), stop=False,
                        )
                    lhsT = ycsb[:, t * SUB : (t + 1) * SUB]
                    nc.tensor.matmul(
                        ps[:, 0:512], lhsT=lhsT, rhs=wcbsb[:, 0:512],
                        start=False, stop=True,
                    )
                    nc.tensor.matmul(
                        ps[:, 512:1024], lhsT=lhsT, rhs=wcbsb[:, 512:1024],
                        start=False, stop=True,
                    )

                    r = rpool.tile([128, F], _DT.float32, tag="r")
                    nc.vector.tensor_reduce(
                        out=r[:],
                        in_=ps[:].rearrange("p (d f) -> p f d", d=ND),
                        axis=mybir.AxisListType.X,
                        op=mybir.AluOpType.max,
                    )
                    rr = rpool.tile([128, F], _DT.float32, tag="rr")
                    nc.vector.tensor_scalar_max(rr[:], r[:], 0.0)
                    nc.sync.dma_start(out[t * SUB : (t + 1) * SUB, :], rr[:])
                t0 += bs

    nc.compile()
    return nc


def _build_w(kernel):
    """Quad-ordered conv weights [128, NQ*NDF].

    wsb[p, k, n] = Wconv[slot 4k + p//32, channel p%32, n]
    with Wconv[j, c, (d,f)] = kernel[j//16, (j%16 - d) % 16, c, f].
    """
    kernel = np.asarray(kernel, dtype=np.float32)
    jj = np.arange(ND)
    d = np.arange(ND)
    dd = (jj[:, None] - d[None, :]) % ND         # [jj, d]
    wconv = kernel[:, dd, :, :]                  # [NR, jj, d, C, F]
    wconv = wconv.transpose(0, 1, 3, 2, 4).reshape(NSLOT, C, NDF)  # [j, c, n]
    p = np.arange(128)
    wp = np.empty((128, NQ, NDF), dtype=np.float32)
    for k in range(NQ):
        wp[:, k, :] = wconv[4 * k + p // 32, p % 32, :]
    return np.ascontiguousarray(wp.reshape(128, NQ * NDF).astype(BF16))


def _build_wcb(center_kernel, bias):
    wcb = np.empty((33, NDF), dtype=np.float32)
    wcb[:32] = np.broadcast_to(
        np.asarray(center_kernel, np.float32)[:, None, :], (C, ND, F)
    ).reshape(C, NDF)
    wcb[32] = np.broadcast_to(np.asarray(bias, np.float32)[None, :], (ND, F)).reshape(NDF)
    return np.ascontiguousarray(wcb.astype(BF16))


def _build_ytab(yb):
    """HBM gather table [NV, 128] bf16: row u = [y[u] (32ch) | zeros(96)]."""
    t = np.zeros((NV, ES), dtype=BF16)
    t[:, :C] = yb.astype(BF16)
    return np.ascontiguousarray(t)


def _build_ycen(yb, v0):
    """Center/bias operand [33, NPAD]: rows 0-31 own-slab channels, row 32 ones."""
    yc = np.zeros((33, NPAD), dtype=np.float32)
    yc[:32, :VPC] = yb[v0 : v0 + VPC].T
    yc[32, :] = 1.0
    return np.ascontiguousarray(yc.astype(BF16))


def _build_idx(vert_b, v0):
    """Wrapped int16 gather indices [128, idx_cols] for one core's slab.

    Per batch (bs subtiles), per quad call k: flat gather position
    i = (t2*4 + g)*128 + p  ->  index vert[v0 + (t0+t2)*128 + p, slot 4k+g],
    wrapped as idx[i%16, ioff + i//16] and replicated across the 8 Q7 groups
    (dma_gather reads partitions 0-15).
    """
    slots = np.zeros((NPAD, NSLOT), dtype=np.int64)
    slots[:VPC] = vert_b[v0 : v0 + VPC].reshape(VPC, NSLOT)
    cols = []
    t0 = 0
    for bs in BATCHES:
        ni = bs * 4 * SUB
        # flat[i] for call k: i = (t2*4+g)*128 + p
        S = slots[t0 * SUB : (t0 + bs) * SUB].reshape(bs, SUB, NSLOT)
        blk = np.empty((NQ, 128, ni // 16), dtype=np.int16)
        for k in range(NQ):
            flat = np.empty(ni, dtype=np.int16)
            for t2 in range(bs):
                for gmem in range(4):
                    r = t2 * 4 + gmem
                    flat[r * SUB : (r + 1) * SUB] = S[t2, :, 4 * k + gmem]
            wrapped = flat.reshape(ni // 16, 16).T          # [16, ni/16]
            blk[k] = np.tile(wrapped, (8, 1))
        cols.append(blk.transpose(1, 0, 2).reshape(128, NQ * (ni // 16)))
        t0 += bs
    return np.ascontiguousarray(np.concatenate(cols, axis=1))


_NC_CACHE = None
_LAST_IN_MAPS = None


def _host_fallback(y, exp_map, kernel, center_kernel, bias):
    """Numpy reference path; only used if exp_map's batch column is nonstandard."""
    patches = y[exp_map[..., 0], exp_map[..., 1]]        # [B, NV, NR, ND, C]
    jj = np.arange(ND)
    d = np.arange(ND)
    wk = kernel[:, (jj[:, None] - d[None, :]) % ND]      # [NR, jj, d, C, F]
    z = np.einsum("bvrjc,rjdcf->bvdf", patches, wk, optimize=True)
    z = z + (y @ center_kernel)[:, :, None, :] + bias
    return np.max(np.maximum(z, 0.0), axis=2).astype(np.float32)


def kernel(y, exp_map, kernel, center_kernel, bias):
    global _NC_CACHE, _LAST_IN_MAPS
    y = np.asarray(y, dtype=np.float32)
    exp_map = np.asarray(exp_map)
    bcast = np.arange(B, dtype=exp_map.dtype)[:, None, None, None]
    if not np.array_equal(exp_map[..., 0], np.broadcast_to(bcast, exp_map.shape[:-1])):
        return _host_fallback(y, exp_map, np.asarray(kernel, np.float32),
                              np.asarray(center_kernel, np.float32),
                              np.asarray(bias, np.float32))
    vert = np.ascontiguousarray(exp_map[..., 1]).astype(np.int64)  # [B, NV, NR, ND]

    wp = _build_w(kernel)
    wcb = _build_wcb(center_kernel, bias)
    tabs = [_build_ytab(y[b]) for b in range(B)]

    in_maps = []
    for core in range(NCORES):
        b = core // (NCORES // B)
        v0 = (core % (NCORES // B)) * VPC
        in_maps.append(
            {
                "ytab": tabs[b],
                "ycen": _build_ycen(y[b], v0),
                "idx": _build_idx(vert[b], v0),
                "w": wp,
                "wcb": wcb,
            }
        )

    if _NC_CACHE is None:
        _NC_CACHE = build_graph()
    nc = _NC_CACHE
    _LAST_IN_MAPS = in_maps

    res = run_bass_kernel_spmd(nc, in_maps, core_ids=list(range(NCORES)))
    outs = [res.results[i]["out"][:VPC] for i in range(NCORES)]
    full = np.concatenate(outs, axis=0).reshape(B, NV, F).astype(np.float32)
    return full


if __name__ == "__main__":
    rng = np.random.default_rng(0)
    y = rng.standard_normal((B, NV, C), dtype=np.float32)
    vert = rng.integers(0, NV, size=(B, NV, NR, ND), dtype=np.int32)
    bidx = np.broadcast_to(np.arange(B, dtype=np.int32)[:, None, None, None], vert.shape)
    exp_map = np.stack([bidx, vert], axis=-1)
    kern = rng.standard_normal((NR, ND, C, F), dtype=np.float32) * 0.05
    ck = rng.standard_normal((C, F), dtype=np.float32) * 0.05
    bs = np.zeros((F,), dtype=np.float32)
    out = kernel(y=y, exp_map=exp_map, kernel=kern, center_kernel=ck, bias=bs)
    print("out", out.shape, out.dtype, float(out.mean()))
